# revision 39
# baseline (speedup 1.0000x reference)
"""BernNetHead Trainium2 kernel (8-core SPMD, channel-sharded).

Math: the Bernstein filter  Hf = sum_i theta[i]*C(K,i) * L^i (I-L)^{K-i} H
collapses (L and I-L commute) into a degree-K monomial polynomial
Hf[:, h] = sum_d c[d, h] * L^d H[:, h], with c computed on the host from
theta/binomials.  The kernel then needs only K=8 sequential applications
of L to H instead of 16 N^3 power matmuls.

Sharding: HID=128 channels split 16-per-core across 8 cores.  BatchNorm
statistics, the polynomial chain, activations and pooling are all
channel-local, so there are no collectives.  Each core returns a [128,1]
hidden-pool slice and a [4,10] partial-logits tensor; the host sums the
partials and concatenates the slices.

On-chip layout: activations live in a transposed [c, n] orientation where
c = b*32 + ch (ch<16 real, 16..31 zero padding so matmul outputs fill
32-partition groups).  A chain step computes V_{d+1}[c, n] with U_d as the
128-col stationary operand and L (symmetric) as the float32r moving
operand at free=512 (full-rate fp32), then 8 PE transposes restore the
[n, c] stationary layout for the next step.
"""

import math

import numpy as np

import concourse.bacc as bacc
import concourse.mybir as mybir
import concourse.tile as tile
from concourse import bass_utils

B, N, F0, HID, OUT, K = 4, 1024, 256, 128, 10, 8
NCORES = 8
CPC = HID // NCORES  # 16 real channels per core
EPS = 1e-5
F32 = mybir.dt.float32
F32R = mybir.dt.float32r
F16 = mybir.dt.float16

# smalls tensor column offsets ([128, SMW] fp32)
PMAT = 0       # 128: BN collect+broadcast matrix,
               # pmat[j,j'] = (both valid, j%32==j'%32)/(B*N)
BMASK = 144    # 4: bmask[j, b] = (j//32 == b and j%32 < 16)
W2SEL = 148    # 10: fc2_w[o, h(j)]/N  (0 on pad rows)
M2SEL = 158    # 10: mlp2_w[o, h(j)]/N
CD = 168       # 9: monomial coeffs c[d, h(j)]
BNP = 177      # 6: [g1, b1, gm, bm, g2, b2] at c-layout
IDENT = 192    # 128: identity matrix for PE transposes
WTS = 448      # 128 f32 cols = [128, 256] f16: fc1/mlp1 weights + f16 identity
XTP = 576      # 4096 f32 cols = [128, 8192] f16: x transposed/packed
SMW = 4672

_NC_CACHE = None


def _r(ap):
    return ap.bitcast(F32R)


def _emit_bn_stats(nc, sc, stat_pool, src, pmat, g_ap, b_ap, tag,
                   per_b=False):
    """BN over the free dim + partition b-groups of src [128, 1024].

    Returns (scale_ap, bias_ap) as per-partition [128,1] columns s.t.
    normalized = src*scale + bias.  per_b=True emits the reduce/square in
    32-partition slices so each starts as soon as its fc copy lands."""
    stat1 = stat_pool.tile([128, 2], F32, tag=f"stat1_{tag}")
    sq = stat_pool.tile([128, 1024], F32, tag="sq_scratch")
    if per_b:
        for b in range(B):
            pr = slice(b * 32, b * 32 + 32)
            nc.vector.reduce_sum(stat1[pr, 0:1], src[pr, :],
                                 axis=mybir.AxisListType.X)
            nc.scalar.activation(sq[pr, :], src[pr, :],
                                 mybir.ActivationFunctionType.Square,
                                 accum_out=stat1[pr, 1:2])
    else:
        nc.vector.reduce_sum(stat1[:, 0:1], src[:], axis=mybir.AxisListType.X)
        nc.scalar.activation(sq[:], src[:],
                             mybir.ActivationFunctionType.Square,
                             accum_out=stat1[:, 1:2])
    return _bn_from_stat1(nc, sc, stat_pool, stat1, pmat, g_ap, b_ap, tag)


def _bn_from_stat1(nc, sc, stat_pool, stat1, pmat, g_ap, b_ap, tag):
    # one matmul collects the per-channel (b,n) sums AND broadcasts them
    # back to all (b, ch) partitions, pre-scaled by 1/(B*N):
    # bc[j, :] = [mean, E[x^2]] of channel(j)
    bc_ps = sc["spsum"].tile([128, 2], F32, tag="sp")
    nc.tensor.matmul(bc_ps[:], pmat, stat1[:])
    bc = stat_pool.tile([128, 8], F32, tag=f"bc_sb_{tag}")
    nc.vector.tensor_copy(bc[:, 0:2], bc_ps[:])
    nc.vector.tensor_mul(bc[:, 2:3], bc[:, 0:1], bc[:, 0:1])   # mean^2
    nc.vector.tensor_sub(bc[:, 2:3], bc[:, 1:2], bc[:, 2:3])   # var
    nc.vector.tensor_scalar_add(bc[:, 2:3], bc[:, 2:3], EPS)
    nc.scalar.activation(bc[:, 3:4], bc[:, 2:3],
                         mybir.ActivationFunctionType.Sqrt)
    nc.vector.reciprocal(bc[:, 4:5], bc[:, 3:4])               # 1/sqrt(var+eps)
    # scale = g*inv ; bias = b - mean*scale
    nc.vector.tensor_mul(bc[:, 5:6], g_ap, bc[:, 4:5])
    nc.vector.tensor_mul(bc[:, 6:7], bc[:, 0:1], bc[:, 5:6])
    nc.vector.tensor_sub(bc[:, 7:8], b_ap, bc[:, 6:7])
    return bc[:, 5:6], bc[:, 7:8]


def _emit_kernel(nc, tc, tensors=None):
    if tensors is None:
        sm_d = nc.dram_tensor("smalls", [128, SMW], F32,
                              kind="ExternalInput").ap()
        l_d = nc.dram_tensor("lmat", [N, N], F16, kind="ExternalInput").ap()
        out_d = nc.dram_tensor("outp", [128, 8], F32,
                               kind="ExternalOutput").ap()
    else:
        sm_d, l_d, out_d = tensors

    import contextlib
    with contextlib.ExitStack() as ctx:
        const = ctx.enter_context(tc.tile_pool(name="const", bufs=1))
        act = ctx.enter_context(tc.tile_pool(name="act", bufs=1))
        stat_pool = ctx.enter_context(tc.tile_pool(name="stats", bufs=1))
        upool = ctx.enter_context(tc.tile_pool(name="upool", bufs=2))
        vspool = ctx.enter_context(tc.tile_pool(name="vspool", bufs=2))
        fc_ps = ctx.enter_context(
            tc.tile_pool(name="fc_ps", bufs=1, space="PSUM"))
        v_ps = ctx.enter_context(
            tc.tile_pool(name="v_ps", bufs=3, space="PSUM"))
        t_ps = ctx.enter_context(
            tc.tile_pool(name="t_ps", bufs=1, space="PSUM"))
        spsum = ctx.enter_context(
            tc.tile_pool(name="spsum", bufs=1, space="PSUM"))
        sc = {"spsum": spsum}

        # ---- loads: just TWO input DMAs (per-DMA overhead on this
        # runtime is ~7us, so everything is packed into one constants+x
        # tensor plus the L matrix) ----
        sm = const.tile([128, SMW], F32)
        nc.sync.dma_start(sm[:], sm_d[:])
        lsb = const.tile([128, 8 * N], F16)
        nc.sync.dma_start(lsb[:],
                          l_d.rearrange("(mi p) n -> p mi n", p=128))
        wts = sm[:, WTS:WTS + 128].bitcast(F16)
        xtiles = {(ft, b): sm[:, XTP + (ft * B + b) * 512:
                              XTP + (ft * B + b) * 512 + 512].bitcast(F16)
                  for ft in range(2) for b in range(B)}
        ltiles = [lsb[:, mi * N:(mi + 1) * N] for mi in range(8)]

        pmat = sm[:, PMAT:PMAT + 128]
        ident = sm[:, IDENT:IDENT + 128]
        cd = sm[:, CD:CD + (K + 1)]

        # preload the ACT function table containing Sqrt (one-time ~1.3us)
        # off the critical path, and a zeros tile for DVE-side relu
        warm = stat_pool.tile([128, 2], F32, tag="warm")
        zeros = stat_pool.tile([128, 512], F32, tag="zeros")
        nc.vector.memset(zeros[:], 0.0)
        nc.vector.memset(warm[:, 0:1], 1.0)
        nc.scalar.activation(warm[:, 1:2], warm[:, 0:1],
                             mybir.ActivationFunctionType.Sqrt)

        # ---- fc1 / mlp1: out[c=(b,ch), n] via 32-padded channel groups ----
        # (fp32r matmuls require dst partition base 0 -> per-b psum tiles).
        # Both streams interleaved per b so each arriving x chunk feeds 8
        # matmuls, halving the DMA-starvation gaps.
        vh = act.tile([128, N], F32, tag="vh")
        zm = act.tile([128, N], F32, tag="zm")
        for b in range(B):
            for st_i, woff, dst in ((0, 0, vh), (1, 64, zm)):
                ps = fc_ps.tile([32, N], F32, tag="fcp")
                for nch in range(2):
                    for ft in range(2):
                        nc.tensor.matmul(
                            ps[:, nch * 512:nch * 512 + 512],
                            wts[:, woff + ft * 32:woff + ft * 32 + 32],
                            xtiles[(ft, b)][:, nch * 512:nch * 512 + 512],
                            start=(ft == 0), stop=(ft == 1))
                eng = (nc.scalar.copy if st_i == 0
                       else nc.vector.tensor_copy)
                eng(dst[b * 32:b * 32 + 32, :], ps[:])

        # ---- BN1 on H, then in-place apply (vh := BN1(H)) ----
        s1, t1 = _emit_bn_stats(nc, sc, stat_pool, vh, pmat,
                                sm[:, BNP + 0:BNP + 1], sm[:, BNP + 1:BNP + 2],
                                "bn1")
        nc.scalar.activation(vh[:, 0:512], vh[:, 0:512],
                             mybir.ActivationFunctionType.Identity,
                             scale=s1, bias=t1)
        nc.vector.tensor_scalar(vh[:, 512:1024], vh[:, 512:1024], s1, t1,
                                op0=mybir.AluOpType.mult,
                                op1=mybir.AluOpType.add)

        # ---- mlp stream: BN + relu + pool in one activation ----
        sm_s, sm_t = _emit_bn_stats(nc, sc, stat_pool, zm, pmat,
                                    sm[:, BNP + 2:BNP + 3],
                                    sm[:, BNP + 3:BNP + 4], "mbn")
        zsc = stat_pool.tile([128, 1024], F32, tag="sq_scratch")
        zsums = stat_pool.tile([128, 2], F32, tag="zsums")
        nc.scalar.activation(zsc[:], zm[:], mybir.ActivationFunctionType.Relu,
                             scale=sm_s, bias=sm_t, accum_out=zsums[:, 0:1])

        # ---- polynomial chain ----
        hft = act.tile([128, N], F32, tag="hft")
        nc.vector.tensor_scalar_mul(hft[:], vh[:], cd[:, 0:1])

        src = vh  # [c, n] V-form input of round d (V_{d-1})
        for d in range(1, K + 1):
            # transpose src [c, n] -> U_{d-1} [n, c] (8 PE transposes; fp32
            # for the first round's f32 vh, f32r after)
            sdt = F32 if d == 1 else F16
            tpa = t_ps.tile([128, 512], sdt, tag="tpa")
            tpb = t_ps.tile([128, 512], sdt, tag="tpb")
            idt = ident if d == 1 else wts[:, 128:256]
            ucur = upool.tile([128, N], F16, tag="u")
            for half, tp in ((0, tpa), (1, tpb)):
                for ni in range(4):
                    nj = half * 4 + ni
                    nc.tensor.transpose(tp[:, ni * 128:(ni + 1) * 128],
                                        src[:, nj * 128:(nj + 1) * 128], idt)
                eng = nc.scalar.copy if half == 0 else nc.vector.tensor_copy
                eng(ucur[:, half * 512:half * 512 + 512], tp[:])
            # V_d = (L U_{d-1})^T : U stationary, L moving (f32r, free=512).
            # Round 1 runs mi-outer so compute follows L-tile DMA arrival.
            vp0 = v_ps.tile([128, 512], F32, tag="vp")
            vp1 = v_ps.tile([128, 512], F32, tag="vp")
            vps = [vp0, vp1]
            if d < K:
                vs = vspool.tile([128, N], F16, tag="vs")
            else:
                vs = None
            if d == 1:
                for mi in range(8):
                    for nch in range(2):
                        nc.tensor.matmul(
                            vps[nch][:],
                            ucur[:, mi * 128:(mi + 1) * 128],
                            ltiles[mi][:, nch * 512:nch * 512 + 512],
                            start=(mi == 0), stop=(mi == 7))
            else:
                for nch in range(2):
                    for mi in range(8):
                        nc.tensor.matmul(
                            vps[nch][:],
                            ucur[:, mi * 128:(mi + 1) * 128],
                            ltiles[mi][:, nch * 512:nch * 512 + 512],
                            start=(mi == 0), stop=(mi == 7))
                    if vs is not None:
                        # drain this psum bank while PE streams the other
                        e0, e1 = ((nc.scalar.copy, nc.vector.tensor_copy)
                                  if nch == 0 else
                                  (nc.vector.tensor_copy, nc.scalar.copy))
                        e0(vs[:, nch * 512:nch * 512 + 256],
                           vps[nch][:, 0:256])
                        e1(vs[:, nch * 512 + 256:nch * 512 + 512],
                           vps[nch][:, 256:512])
            if d < K:
                if d == 1:
                    nc.scalar.copy(vs[:, 0:512], vps[0][:])
                    nc.vector.tensor_copy(vs[:, 512:1024], vps[1][:])
                for nch in range(2):
                    nc.vector.scalar_tensor_tensor(
                        hft[:, nch * 512:(nch + 1) * 512], vps[nch][:],
                        cd[:, d:d + 1],
                        hft[:, nch * 512:(nch + 1) * 512],
                        op0=mybir.AluOpType.mult, op1=mybir.AluOpType.add)
                src = vs
            else:
                # last round: FMA from PSUM, fusing the BN2 free-dim sums
                # via accum_out; squared sums per half right after
                st2 = stat_pool.tile([128, 8], F32, tag="st2")
                sqh = stat_pool.tile([128, 1024], F32, tag="sq_scratch")
                for nch in range(2):
                    sl = slice(nch * 512, nch * 512 + 512)
                    nc.vector.scalar_tensor_tensor(
                        hft[:, sl], vps[nch][:], cd[:, d:d + 1], hft[:, sl],
                        op0=mybir.AluOpType.mult, op1=mybir.AluOpType.add,
                        accum_out=st2[:, nch:nch + 1])
                    nc.scalar.activation(
                        sqh[:, sl], hft[:, sl],
                        mybir.ActivationFunctionType.Square,
                        accum_out=st2[:, 4 + nch:5 + nch])

        # ---- BN2 on Hf (stats pre-collected in st2), residual relu, pool ----
        stat2 = stat_pool.tile([128, 2], F32, tag="stat2")
        nc.vector.tensor_add(stat2[:, 0:1], st2[:, 0:1], st2[:, 1:2])
        nc.vector.tensor_add(stat2[:, 1:2], st2[:, 4:5], st2[:, 5:6])
        s2, t2 = _bn_from_stat1(nc, sc, stat_pool, stat2, pmat,
                                sm[:, BNP + 4:BNP + 5], sm[:, BNP + 5:BNP + 6],
                                "bn2")
        hhf = stat_pool.tile([128, 1024], F32, tag="hhf")
        psc = stat_pool.tile([128, 1024], F32, tag="sq_scratch")
        psums = stat_pool.tile([128, 4], F32, tag="psums")
        # h0: DVE residual-add then ACT relu+pool; h1: both on DVE (relu as
        # (x+t) max 0 via tensor_scalar) so the halves run in parallel
        nc.vector.scalar_tensor_tensor(hhf[:, 0:512], hft[:, 0:512], s2,
                                       vh[:, 0:512],
                                       op0=mybir.AluOpType.mult,
                                       op1=mybir.AluOpType.add)
        nc.scalar.activation(psc[:, 0:512], hhf[:, 0:512],
                             mybir.ActivationFunctionType.Relu,
                             bias=t2, accum_out=psums[:, 0:1])
        nc.vector.scalar_tensor_tensor(hhf[:, 512:1024], hft[:, 512:1024], s2,
                                       vh[:, 512:1024],
                                       op0=mybir.AluOpType.mult,
                                       op1=mybir.AluOpType.add)
        nc.scalar.activation(psc[:, 512:1024], hhf[:, 512:1024],
                             mybir.ActivationFunctionType.Relu,
                             bias=t2, accum_out=psums[:, 1:2])
        nc.vector.tensor_add(psums[:, 2:3], psums[:, 0:1], psums[:, 1:2])

        # ---- outputs: packed into one [128, 8] tensor / one DMA ----
        out_sb = stat_pool.tile([128, 8], F32, tag="out_sb")
        nc.vector.memset(out_sb[:], 0.0)
        hid_sb = stat_pool.tile([128, 2], F32, tag="hid_sb")
        nc.vector.tensor_add(hid_sb[:, 0:1], psums[:, 2:3], zsums[:, 0:1])
        nc.scalar.mul(out_sb[:, 0:1], hid_sb[:, 0:1], 0.5 / N)

        pmask = stat_pool.tile([128, 8], F32, tag="pmask")
        nc.vector.tensor_scalar_mul(pmask[:, 0:4], sm[:, BMASK:BMASK + 4],
                                    psums[:, 2:3])
        nc.vector.tensor_scalar_mul(pmask[:, 4:8], sm[:, BMASK:BMASK + 4],
                                    zsums[:, 0:1])
        lg_ps = spsum.tile([OUT, B], F32, tag="sp")
        nc.tensor.matmul(lg_ps[:], sm[:, W2SEL:W2SEL + OUT], pmask[:, 0:4],
                         start=True, stop=False)
        nc.tensor.matmul(lg_ps[:], sm[:, M2SEL:M2SEL + OUT], pmask[:, 4:8],
                         start=False, stop=True)
        nc.vector.tensor_copy(out_sb[0:OUT, 4:8], lg_ps[:])
        nc.sync.dma_start(out_d[:], out_sb[:])


def build_nc(reps=1):
    global _NC_CACHE
    if _NC_CACHE is not None and reps == 1:
        return _NC_CACHE
    nc = bacc.Bacc("TRN2", target_bir_lowering=False, debug=False,
                   enable_asserts=False, num_devices=NCORES)
    tensors = None
    if reps > 1:
        tensors = (
            nc.dram_tensor("smalls", [128, SMW], F32,
                           kind="ExternalInput").ap(),
            nc.dram_tensor("lmat", [N, N], F16, kind="ExternalInput").ap(),
            nc.dram_tensor("outp", [128, 8], F32,
                           kind="ExternalOutput").ap(),
        )
    with tile.TileContext(nc) as tc:
        if reps > 1:
            with tc.For_i(0, reps, 1):
                _emit_kernel(nc, tc, tensors)
        else:
            _emit_kernel(nc, tc, tensors)
    nc.compile()
    if reps == 1:
        _NC_CACHE = nc
    return nc


def _monomial_coeffs(theta):
    """c[d, h] = sum_{i<=d} C(K,i) theta[i,h] C(K-i, d-i) (-1)^{d-i}."""
    th = np.asarray(theta, dtype=np.float64)
    c = np.zeros((K + 1, HID))
    for d in range(K + 1):
        for i in range(d + 1):
            c[d] += (math.comb(K, i) * math.comb(K - i, d - i)
                     * ((-1.0) ** (d - i)) * th[i])
    return c.astype(np.float32)


def make_smalls(fc1_w, mlp1_w, bn1_g, bn1_b, mbn_g, mbn_b, bn2_g, bn2_b,
                theta, fc2_w, mlp2_w):
    """Per-core [128, SMW] parameter/constant blocks."""
    c = _monomial_coeffs(theta)
    out = []
    j = np.arange(128)
    g = j % 32
    b = j // 32
    valid = g < CPC
    for k in range(NCORES):
        sl = np.zeros((128, SMW), np.float32)
        hsel = np.where(valid, k * CPC + np.minimum(g, CPC - 1), 0)
        # fused BN collect+broadcast matrix (symmetric, pre-scaled)
        pm = ((valid[:, None] & valid[None, :])
              & (g[:, None] == g[None, :])).astype(np.float32) / (B * N)
        sl[:, PMAT:PMAT + 128] = pm
        for bb in range(B):
            sl[:, BMASK + bb] = ((b == bb) & valid).astype(np.float32)
        sl[:, W2SEL:W2SEL + OUT] = np.where(
            valid[:, None], fc2_w[:, hsel].T / N, 0.0)
        sl[:, M2SEL:M2SEL + OUT] = np.where(
            valid[:, None], mlp2_w[:, hsel].T / N, 0.0)
        sl[:, CD:CD + K + 1] = np.where(valid[:, None], c[:, hsel].T, 0.0)
        for col, arr in enumerate((bn1_g, bn1_b, mbn_g, mbn_b, bn2_g, bn2_b)):
            sl[:, BNP + col] = np.where(valid, arr[hsel], 0.0)
        sl[:, IDENT:IDENT + 128] = np.eye(128, dtype=np.float32)
        wt = np.zeros((128, 256), np.float16)
        for ft in range(2):
            w = fc1_w[k * CPC:(k + 1) * CPC, ft * 128:(ft + 1) * 128]
            wt[:, ft * 32:ft * 32 + CPC] = w.T.astype(np.float16)
            w = mlp1_w[k * CPC:(k + 1) * CPC, ft * 128:(ft + 1) * 128]
            wt[:, 64 + ft * 32:64 + ft * 32 + CPC] = w.T.astype(np.float16)
        wt[:, 128:256] = np.eye(128, dtype=np.float16)
        sl[:, WTS:WTS + 128] = wt.view(np.float32)
        out.append(sl)
    return out


def make_in_maps(x_in, L, **weights):
    # x packed as [p, (ft, b, n)] fp16 and embedded into the smalls tensor
    xt = np.asarray(x_in, np.float32).transpose(2, 0, 1).astype(np.float16)
    xp = np.ascontiguousarray(
        xt.reshape(2, 128, B * N).transpose(1, 0, 2).reshape(128, 2 * B * N))
    lm = np.ascontiguousarray(np.asarray(L, np.float32)).astype(np.float16)
    smalls = make_smalls(**{k: np.asarray(v, np.float32)
                            for k, v in weights.items()})
    maps = []
    for k in range(NCORES):
        sl = smalls[k]
        sl[:, XTP:XTP + 4096] = xp.view(np.float32)
        maps.append({"smalls": sl, "lmat": lm})
    return maps


def unshard(results, fc2_b, mlp2_b):
    hp = np.stack([r["outp"][:, 0] for r in results])       # [8, 128]
    hidden = hp.reshape(NCORES, B, 32)[:, :, :CPC].transpose(1, 0, 2)
    hidden = np.ascontiguousarray(hidden.reshape(B, HID), dtype=np.float32)
    logits = np.sum([r["outp"][0:OUT, 4:4 + B].T for r in results], axis=0)
    logits = (logits + np.asarray(fc2_b, np.float32)
              + np.asarray(mlp2_b, np.float32)).astype(np.float32)
    return hidden, logits


def kernel(x_in, L, fc1_w, fc1_b, bn1_g, bn1_b, bn2_g, bn2_b, theta,
           fc2_w, fc2_b, mlp1_w, mlp1_b, mbn_g, mbn_b, mlp2_w, mlp2_b,
           dropout_p, **_unused):
    nc = build_nc()
    in_maps = make_in_maps(
        x_in, L, fc1_w=fc1_w, mlp1_w=mlp1_w, bn1_g=bn1_g, bn1_b=bn1_b,
        mbn_g=mbn_g, mbn_b=mbn_b, bn2_g=bn2_g, bn2_b=bn2_b, theta=theta,
        fc2_w=fc2_w, mlp2_w=mlp2_w)
    res = bass_utils.run_bass_kernel_spmd(nc, in_maps,
                                          core_ids=list(range(NCORES)))
    return unshard(res.results, fc2_b, mlp2_b)
